# Initial kernel scaffold
#
"""Trainium2 Bass kernel for the CIN (Compressed Interaction Network) problem.

Reference computation (per full batch B=1024, F=39, D=32):
    y1 = relu(einsum('bfd,bgd,ofg', x, x, w1) + b1)        # (B, 400, D)
    x1, h1 = y1[:, :200], y1[:, 200:]
    y2 = relu(einsum('bfd,bgd,ofg', x, h1, w2) + b2)
    x2, h2 = y2[:, :200], y2[:, 200:]
    y3 = relu(einsum('bfd,bgd,ofg', x, h2, w3) + b3)
    pooled = concat([x1, x2, y3], 1).sum(-1)               # (B, 800)
    out = pooled @ out_w + out_b                           # (B, 400)
    out = batchnorm_train(out, gamma, beta); relu

Strategy: data-parallel over batch (128 rows/core on 8 cores). Each CIN layer
is a GEMM over tokens t=(b,d): z[t,o] = sum_{f,g} V[(f,g),t] * W[(f,g),o]
with V^T[(f,g),t] = x^T[f,t]*h^T[g,t] built on-the-fly on the vector engine
(bf16, fp32 PSUM accumulation). x-side replication tiles are precomputed on
host; h^T comes from PE transposes of each layer's output. Final BN stats are
all-reduced across cores with a tiny collective.
"""

import sys

sys.path.insert(0, "/opt/trn_rl_repo")

import numpy as np
import ml_dtypes

import concourse.bass as bass
import concourse.bacc as bacc
import concourse.mybir as mybir
import concourse.tile as tile
from concourse.masks import make_identity
from concourse.bass_utils import run_bass_kernel_spmd

BF16 = ml_dtypes.bfloat16
FP32 = mybir.dt.float32
BF = mybir.dt.bfloat16

N_CORES = 8
B, F, D = 1024, 39, 32
CL, HALF, OUT = 400, 200, 400
FINAL = 800
EPS = 1e-5

B_LOC = B // N_CORES          # 128 batch rows per core
T_TOT = B_LOC * D             # 4096 tokens per core
TBD = 512                     # tokens per bd-tile
N_BD = T_TOT // TBD           # 8 bd-tiles
N_MS = TBD // 128             # 4 m-slices per bd-tile
NB_TILE = TBD // D            # 16 batch rows per bd-tile

# L1 K-chunking: 13 chunks of 117 rows (3 f's x 39 g's each), K1 = 1521
L1_CHUNKS = 13
L1_ROWS = 117
# L2/L3 K-chunking: per f: A chunk (g 0..127) + B chunk (g 128..199)
GA, GB = 128, 72

# pooled channel chunks: (tag, layer_part, ch0, length, global_row0)
# pooled order = [x1(200) | x2(200) | y3(400)]
POOL_CHUNKS = [
    (0, 1, 0, 128), (1, 1, 128, 72),
    (2, 2, 0, 128), (3, 2, 128, 72),
    (4, 3, 0, 128), (5, 3, 128, 128), (6, 3, 256, 128), (7, 3, 384, 16),
]
POOL_ROW0 = [0, 128, 200, 328, 400, 528, 656, 784]
POOL_LEN = [128, 72, 128, 72, 128, 128, 128, 16]


def _host_prep(x_shard, w1, b1, w2, b2, w3, b3, out_w, out_b):
    """Pure layout prep (transpose/tile/cast) of one core's inputs."""
    # x^T: (F, T) with token t = b*D + d
    xT = np.ascontiguousarray(x_shard.transpose(1, 0, 2).reshape(F, T_TOT))
    # x3[p] = xT[p % 39]  (in0 for L1/L2/L3 f-runs)  -> [N_BD, 117, TBD]
    x3 = np.tile(xT, (3, 1))[:L1_ROWS]
    x3 = np.ascontiguousarray(
        x3.reshape(L1_ROWS, N_BD, TBD).transpose(1, 0, 2)).astype(BF16)
    # xrep[j, p] = xT[3j + p//39]  (L1 in1)  -> [N_BD, 13, 117, TBD]
    xrep = np.empty((L1_CHUNKS, L1_ROWS, T_TOT), np.float32)
    for j in range(L1_CHUNKS):
        for q in range(3):
            xrep[j, 39 * q:39 * (q + 1)] = xT[3 * j + q][None, :]
    xrep = np.ascontiguousarray(
        xrep.reshape(L1_CHUNKS, L1_ROWS, N_BD, TBD).transpose(2, 0, 1, 3)
    ).astype(BF16)
    # repx[f] = xT[f] broadcast to 128 partitions  -> [N_BD, 39, 128, TBD]
    repx = np.broadcast_to(xT[:, None, :], (F, 128, T_TOT))
    repx = np.ascontiguousarray(
        repx.reshape(F, 128, N_BD, TBD).transpose(2, 0, 1, 3)).astype(BF16)

    # weights, K=(f,g) flattened f-major
    w1r = np.ascontiguousarray(
        w1.transpose(1, 2, 0).reshape(F * F, CL)).astype(BF16)
    def _wpack(w):
        wr = w.transpose(1, 2, 0).reshape(F, HALF, CL)  # [f, g, o]
        wp = np.zeros((F, 128, 2 * CL), np.float32)
        wp[:, :, :CL] = wr[:, :GA]
        wp[:, :GB, CL:] = wr[:, GA:]
        return wp.astype(BF16)
    w2r = _wpack(w2)
    w3r = _wpack(w3)
    worc = np.zeros((8, 128, OUT), np.float32)
    for k in range(8):
        worc[k, :POOL_LEN[k]] = out_w[POOL_ROW0[k]:POOL_ROW0[k] + POOL_LEN[k]]
    worc = worc.astype(BF16)
    biases = np.stack([b1, b2, b3, out_b]).astype(BF16)  # [4, 400]
    return dict(x3=x3, xrep=xrep, repx=repx, w1r=w1r, w2r=w2r, w3r=w3r,
                worc=worc, biases=biases)


def _build_bass():
    nc = bacc.Bacc()
    P = {}
    P["x3"] = nc.declare_dram_parameter("x3", [N_BD, L1_ROWS, TBD], BF, isOutput=False)
    P["xrep"] = nc.declare_dram_parameter("xrep", [N_BD, L1_CHUNKS, L1_ROWS, TBD], BF, isOutput=False)
    P["repx"] = nc.declare_dram_parameter("repx", [N_BD, F, 128, TBD], BF, isOutput=False)
    P["w1r"] = nc.declare_dram_parameter("w1r", [F * F, CL], BF, isOutput=False)
    P["w2r"] = nc.declare_dram_parameter("w2r", [F, 128, 2 * CL], BF, isOutput=False)
    P["w3r"] = nc.declare_dram_parameter("w3r", [F, 128, 2 * CL], BF, isOutput=False)
    P["worc"] = nc.declare_dram_parameter("worc", [8, 128, OUT], BF, isOutput=False)
    P["biases"] = nc.declare_dram_parameter("biases", [4, OUT], BF, isOutput=False)
    P["gamma"] = nc.declare_dram_parameter("gamma", [1, OUT], FP32, isOutput=False)
    P["beta"] = nc.declare_dram_parameter("beta", [1, OUT], FP32, isOutput=False)
    out_d = nc.declare_dram_parameter("out", [B_LOC, OUT], FP32, isOutput=True)

    MULT = mybir.AluOpType.mult
    ADD = mybir.AluOpType.add
    SUB = mybir.AluOpType.subtract
    RELU = mybir.ActivationFunctionType.Relu
    COPY = mybir.ActivationFunctionType.Copy
    SQRT = mybir.ActivationFunctionType.Sqrt

    with tile.TileContext(nc) as tc:
        with (
            tc.tile_pool(name="wpool", bufs=1) as wpool,
            tc.tile_pool(name="consts", bufs=1) as consts,
            tc.tile_pool(name="rpool", bufs=3) as rpool,
            tc.tile_pool(name="xpool", bufs=2) as xpool,
            tc.tile_pool(name="vpool", bufs=3) as vpool,
            tc.tile_pool(name="hpool", bufs=2) as hpool,
            tc.tile_pool(name="ypool", bufs=4) as ypool,
            tc.tile_pool(name="spool", bufs=1) as spool,
            tc.tile_pool(name="psy", bufs=5, space="PSUM") as psy,
            tc.tile_pool(name="pst", bufs=2, space="PSUM") as pst,
            tc.tile_pool(name="psp", bufs=1, space="PSUM") as psp,
            tc.tile_pool(name="dram", bufs=1, space="DRAM") as dram,
        ):
            # ---- constants ----
            ident = consts.tile([128, 128], BF, tag="ident", name="ident")
            make_identity(nc, ident)
            ones_bf = consts.tile([1, 128], BF, tag="ones_bf", name="ones_bf")
            nc.vector.memset(ones_bf[:], 1.0)
            ones_col = consts.tile([128, 1], FP32, tag="ones_col", name="ones_col")
            nc.vector.memset(ones_col[:], 1.0)
            ones_row_f = consts.tile([1, 128], FP32, tag="ones_row_f", name="ones_row_f")
            nc.vector.memset(ones_row_f[:], 1.0)
            # A4[t, j] = 1 if t//32 == j (d-sum aggregation within an m-slice)
            a4 = consts.tile([128, 4], BF, tag="a4", name="a4")
            nc.vector.memset(a4[:], 0.0)
            for j in range(4):
                nc.vector.memset(a4[32 * j:32 * (j + 1), j:j + 1], 1.0)
            gamma_sb = consts.tile([1, OUT], FP32, tag="gamma", name="gamma")
            nc.sync.dma_start(gamma_sb[:], P["gamma"][:])
            beta_sb = consts.tile([1, OUT], FP32, tag="beta", name="beta")
            nc.sync.dma_start(beta_sb[:], P["beta"][:])
            brow = []
            for l in range(4):
                t = consts.tile([1, OUT], BF, tag=f"brow{l}", name=f"brow{l}")
                nc.sync.dma_start(t[:], P["biases"][l:l + 1, :])
                brow.append(t)

            w2c, w3c = [], []

            # pooled^T accumulator chunks, filled 16 b-columns per bd-tile
            pooledT = []
            for k in range(8):
                t = spool.tile([128, B_LOC], BF, tag=f"pooledT{k}", name=f"pooledT{k}")
                pooledT.append(t)

            # ---- main loop over bd-tiles ----
            for c in range(N_BD):
                # x-side tiles for this bd-tile
                x3c = xpool.tile([L1_ROWS, TBD], BF, tag="x3c", name="x3c", bufs=1)
                nc.sync.dma_start(x3c[:], P["x3"][c])
                xrepc = []
                XQ = (1, 6, 6)
                j0 = 0
                for q, nj in enumerate(XQ):
                    t = xpool.tile([L1_ROWS, 6 * TBD], BF, tag="xrepc",
                                   name=f"xrepc{q}", bufs=3)
                    src_ap = P["xrep"][c, j0:j0 + nj].rearrange("j p t -> p j t")
                    dst_ap = t[:, 0:nj * TBD].rearrange("p (j t) -> p j t", j=nj)
                    nc.sync.dma_start(dst_ap, src_ap)
                    for j in range(nj):
                        xrepc.append(t[:, j * TBD:(j + 1) * TBD])
                    j0 += nj
                pooled_ps = psp.tile([128, 128], FP32, tag="pooled", name="pooled")
                w1c = []
                for j in range(L1_CHUNKS):
                    t = wpool.tile([L1_ROWS, CL], BF, tag="w1s", name=f"w1s{c}_{j}", bufs=5)
                    nc.sync.dma_start(t[:], P["w1r"][L1_ROWS * j:L1_ROWS * (j + 1), :])
                    w1c.append(t)

                hT_A = hT_B = None
                for layer in (1, 2, 3):
                    # --- build V^T chunks and accumulate the layer GEMM ---
                    psum_y = [psy.tile([128, CL], FP32, tag="psy", name=f"psy{layer}_{c}_{m}") for m in range(N_MS)]
                    if layer == 1:
                        nck = L1_CHUNKS
                        for j in range(nck):
                            vt = vpool.tile([L1_ROWS, TBD], BF, tag="vt", name="vt")
                            nc.vector.tensor_tensor(vt[:], x3c[:], xrepc[j], MULT)
                            for m in range(N_MS):
                                nc.tensor.matmul(
                                    psum_y[m][:], vt[:, 128 * m:128 * (m + 1)],
                                    w1c[j][:], start=(j == 0), stop=False)
                    else:
                        wc = w2c if layer == 2 else w3c
                        repq = []
                        for q in range((F + 3) // 4):
                            nf = min(4, F - 4 * q)
                            t = rpool.tile([128, 4 * TBD], BF, tag="repc",
                                           name=f"repc{layer}_{q}")
                            src_ap = P["repx"][c, 4 * q:4 * q + nf].rearrange(
                                "f p t -> p f t")
                            dst_ap = t[:, 0:nf * TBD].rearrange(
                                "p (f t) -> p f t", f=nf)
                            nc.sync.dma_start(dst_ap, src_ap)
                            repq.append(t)
                        repc = [repq[f // 4][:, (f % 4) * TBD:(f % 4 + 1) * TBD]
                                for f in range(F)]
                        for f in range(F):
                            if c == 0:
                                # lazy, interleaved first-use weight loads
                                wl = P["w2r"] if layer == 2 else P["w3r"]
                                wp = "w2" if layer == 2 else "w3"
                                tw = wpool.tile([128, 2 * CL], BF, tag=f"{wp}_{f}", name=f"{wp}_{f}")
                                nc.sync.dma_start(tw[:], wl[f])
                                wc.append(tw)
                            va = vpool.tile([GA, TBD], BF, tag="vt", name="vt")
                            nc.vector.tensor_tensor(
                                va[:], hT_A[:], repc[f], MULT)
                            for m in range(N_MS):
                                nc.tensor.matmul(
                                    psum_y[m][:], va[:, 128 * m:128 * (m + 1)],
                                    wc[f][:, 0:CL], start=(f == 0), stop=False)
                            vb = vpool.tile([GB, TBD], BF, tag="vtb", name="vtb")
                            nc.vector.tensor_tensor(
                                vb[:], hT_B[0:GB, :], repc[f][0:GB, 0:TBD], MULT)
                            for m in range(N_MS):
                                nc.tensor.matmul(
                                    psum_y[m][:], vb[:, 128 * m:128 * (m + 1)],
                                    wc[f][0:GB, CL:2 * CL], start=False, stop=False)
                    # bias row (K=1 matmul), closes the accumulation group
                    for m in range(N_MS):
                        nc.tensor.matmul(
                            psum_y[m][:], ones_bf[:], brow[layer - 1][:],
                            start=False, stop=True)

                    # --- drain with relu; produce h^T for next layer ---
                    if layer < 3:
                        hT_A = hpool.tile([GA, TBD], BF, tag="hTa", name="hTa")
                        hT_B = hpool.tile([GB, TBD], BF, tag="hTb", name="hTb")
                    y_sb = []
                    for m in range(N_MS):
                        y = ypool.tile([128, CL], BF, tag="ysb", name="ysb")
                        nc.scalar.activation(y[:], psum_y[m][:], RELU)
                        y_sb.append(y)
                        if layer < 3:
                            ta = pst.tile([128, 128], BF, tag="tp", name="tp")
                            nc.tensor.transpose(
                                ta[:], y[:, HALF:HALF + GA], ident[:])
                            nc.scalar.copy(
                                hT_A[:, 128 * m:128 * (m + 1)], ta[:])
                            tb = pst.tile([GB, 128], BF, tag="tp", name="tp")
                            nc.tensor.transpose(
                                tb[:], y[:, HALF + GA:CL], ident[:])
                            nc.scalar.copy(
                                hT_B[:, 128 * m:128 * (m + 1)], tb[:])
                    # --- pooled d-sums for this layer's x-part / y3 ---
                    for (k, lp, ch0, ln) in POOL_CHUNKS:
                        if lp != layer:
                            continue
                        for m in range(N_MS):
                            nc.tensor.matmul(
                                pooled_ps[0:ln, 16 * k + 4 * m:16 * k + 4 * (m + 1)],
                                y_sb[m][:, ch0:ch0 + ln], a4[:],
                                start=True, stop=True)
                # drain pooled psum into persistent pooled^T chunks
                for k in range(8):
                    ln = POOL_LEN[k]
                    nc.scalar.copy(
                        pooledT[k][0:ln, NB_TILE * c:NB_TILE * (c + 1)],
                        pooled_ps[0:ln, 16 * k:16 * (k + 1)])

                # --- per-bd-tile final GEMM on this tile's 16 batch rows ---
                if c == 0:
                    worc_sb = []
                    for k in range(8):
                        t = wpool.tile([128, OUT], BF, tag=f"wor{k}", name=f"wor{k}")
                        nc.sync.dma_start(t[:], P["worc"][k])
                        worc_sb.append(t)
                    out_sb = spool.tile([B_LOC, OUT], FP32, tag="out_sb", name="out_sb")
                    stats_acc = spool.tile([1, 2 * OUT], FP32, tag="stats_acc", name="stats_acc")
                fo_ps = pst.tile([NB_TILE, OUT], FP32, tag="tp", name=f"fo_ps{c}")
                for k in range(8):
                    nc.tensor.matmul(
                        fo_ps[:], pooledT[k][0:POOL_LEN[k], NB_TILE * c:NB_TILE * (c + 1)],
                        worc_sb[k][0:POOL_LEN[k], :], start=(k == 0), stop=False)
                nc.tensor.matmul(fo_ps[:], ones_bf[0:1, 0:NB_TILE], brow[3][:],
                                 start=False, stop=True)
                o_c = spool.tile([NB_TILE, OUT], FP32, tag="o_c", name="o_c", bufs=1)
                nc.scalar.copy(o_c[:], fo_ps[:])
                nc.sync.dma_start(out_sb[NB_TILE * c:NB_TILE * (c + 1), :], o_c[:])
                sq_c = spool.tile([NB_TILE, OUT], FP32, tag="sq_c", name="sq_c", bufs=1)
                nc.scalar.square(sq_c[:], o_c[:])
                st_ps = pst.tile([1, OUT], FP32, tag="tp", name=f"st_ps{c}")
                nc.tensor.matmul(st_ps[:], ones_col[0:NB_TILE, :], o_c[:],
                                 start=True, stop=True)
                sq_ps = pst.tile([1, OUT], FP32, tag="tp", name=f"sq_ps{c}")
                nc.tensor.matmul(sq_ps[:], ones_col[0:NB_TILE, :], sq_c[:],
                                 start=True, stop=True)
                if c == 0:
                    nc.scalar.copy(stats_acc[0:1, 0:OUT], st_ps[:])
                    nc.scalar.copy(stats_acc[0:1, OUT:2 * OUT], sq_ps[:])
                else:
                    tgt = stats7 if c == N_BD - 1 else stats_acc
                    if c == N_BD - 1:
                        nc.scalar.copy(stats7[0:1, 0:OUT], st_ps[:])
                        nc.scalar.copy(stats7[0:1, OUT:2 * OUT], sq_ps[:])
                    else:
                        nc.vector.tensor_tensor(
                            stats_acc[0:1, 0:OUT], stats_acc[0:1, 0:OUT], st_ps[:], ADD)
                        nc.vector.tensor_tensor(
                            stats_acc[0:1, OUT:2 * OUT], stats_acc[0:1, OUT:2 * OUT],
                            sq_ps[:], ADD)
                if c == N_BD - 2:
                    # early collective over the first 112 rows' stats; its
                    # latency hides under bd-tile 7 compute
                    nc.gpsimd.dma_start(cc_in1[:], stats_acc[:])
                    nc.gpsimd.collective_compute(
                        "AllReduce", ADD,
                        replica_groups=[list(range(N_CORES))],
                        ins=[cc_in1.opt()], outs=[cc_out1.opt()])
                    nc.gpsimd.dma_start(gst1[:], cc_out1[:])
                if c == 0:
                    stats7 = spool.tile([1, 2 * OUT], FP32, tag="stats7", name="stats7")
                    cc_in1 = dram.tile([1, 2 * OUT], FP32, tag="", name="cc_in1")
                    cc_out1 = dram.tile([1, 2 * OUT], FP32, tag="", name="cc_out1")
                    cc_in2 = dram.tile([1, 2 * OUT], FP32, tag="", name="cc_in2")
                    cc_out2 = dram.tile([1, 2 * OUT], FP32, tag="", name="cc_out2")
                    gst1 = spool.tile([1, 2 * OUT], FP32, tag="gst1", name="gst1")

            # ---- second-phase collective: last bd-tile's stats ----
            nc.gpsimd.dma_start(cc_in2[:], stats7[:])
            nc.gpsimd.collective_compute(
                "AllReduce", ADD,
                replica_groups=[list(range(N_CORES))],
                ins=[cc_in2.opt()], outs=[cc_out2.opt()])
            gst = spool.tile([1, 2 * OUT], FP32, tag="stats_acc", name="gst")
            nc.gpsimd.dma_start(gst[:], cc_out2[:])
            nc.vector.tensor_tensor(gst[:], gst[:], gst1[:], ADD)

            # mean/var/scale/shift on partition 0
            mean = spool.tile([1, OUT], FP32, tag="mean", name="mean")
            nc.scalar.mul(mean[:], gst[0:1, 0:OUT], 1.0 / B)
            msq = spool.tile([1, OUT], FP32, tag="scratch", name="msq", bufs=2)
            nc.vector.tensor_tensor(msq[:], mean[:], mean[:], MULT)
            var = spool.tile([1, OUT], FP32, tag="scratch", name="var", bufs=2)
            nc.vector.scalar_tensor_tensor(
                var[:], gst[0:1, OUT:2 * OUT], 1.0 / B, msq[:], MULT, SUB)
            epsc = spool.tile([1, 1], FP32, tag="epsc", name="epsc")
            nc.vector.memset(epsc[:], EPS)
            stdv = spool.tile([1, OUT], FP32, tag="scratch", name="stdv", bufs=2)
            nc.scalar.activation(stdv[:], var[:], SQRT, bias=epsc[:])
            inv = spool.tile([1, OUT], FP32, tag="scratch", name="inv", bufs=2)
            nc.vector.reciprocal(inv[:], stdv[:])
            scal = spool.tile([1, OUT], FP32, tag="scal", name="scal")
            nc.vector.tensor_tensor(scal[:], inv[:], gamma_sb[:], MULT)
            tmp = spool.tile([1, OUT], FP32, tag="scratch", name="tmp", bufs=2)
            nc.vector.scalar_tensor_tensor(
                tmp[:], mean[:], -1.0, scal[:], MULT, MULT)
            shift = spool.tile([1, OUT], FP32, tag="shift", name="shift")
            nc.vector.tensor_tensor(shift[:], tmp[:], beta_sb[:], ADD)

            # broadcast scale/shift across partitions via K=1 matmuls
            bc_scale = psy.tile([128, OUT], FP32, tag="psy", name="psy")
            nc.tensor.matmul(bc_scale[:], ones_row_f[:], scal[:],
                             start=True, stop=True)
            bc_shift = psy.tile([128, OUT], FP32, tag="psy", name="psy")
            nc.tensor.matmul(bc_shift[:], ones_row_f[:], shift[:],
                             start=True, stop=True)
            t1 = spool.tile([128, OUT], FP32, tag="scratch", name="t1", bufs=2)
            nc.vector.tensor_tensor(t1[:], out_sb[:], bc_scale[:], MULT)
            t2 = spool.tile([128, OUT], FP32, tag="scratch", name="t2", bufs=2)
            nc.vector.tensor_tensor(t2[:], t1[:], bc_shift[:], ADD)
            outf = spool.tile([128, OUT], FP32, tag="scratch", name="outf", bufs=2)
            nc.scalar.activation(outf[:], t2[:], RELU)
            nc.sync.dma_start(out_d[:], outf[:])

    nc.compile()
    return nc


_NC_CACHE = None


def kernel(x, w1, b1, w2, b2, w3, b3, out_w, out_b, gamma, beta):
    global _NC_CACHE
    x = np.asarray(x, np.float32)
    shards = [
        _host_prep(x[B_LOC * i:B_LOC * (i + 1)],
                   np.asarray(w1, np.float32), np.asarray(b1, np.float32),
                   np.asarray(w2, np.float32), np.asarray(b2, np.float32),
                   np.asarray(w3, np.float32), np.asarray(b3, np.float32),
                   np.asarray(out_w, np.float32), np.asarray(out_b, np.float32))
        for i in range(N_CORES)
    ]
    gm = np.asarray(gamma, np.float32).reshape(1, OUT)
    bt = np.asarray(beta, np.float32).reshape(1, OUT)
    in_maps = [dict(s, gamma=gm, beta=bt) for s in shards]

    if _NC_CACHE is None:
        _NC_CACHE = _build_bass()
    res = run_bass_kernel_spmd(_NC_CACHE, in_maps, core_ids=list(range(N_CORES)))
    return np.concatenate(
        [res.results[i]["out"] for i in range(N_CORES)], axis=0
    ).astype(np.float32)



# revision 1
# speedup vs baseline: 1.2493x; 1.2493x over previous
"""Trainium2 Bass kernel for the CIN (Compressed Interaction Network) problem.

Reference computation (per full batch B=1024, F=39, D=32):
    y1 = relu(einsum('bfd,bgd,ofg', x, x, w1) + b1)        # (B, 400, D)
    x1, h1 = y1[:, :200], y1[:, 200:]
    y2 = relu(einsum('bfd,bgd,ofg', x, h1, w2) + b2)
    x2, h2 = y2[:, :200], y2[:, 200:]
    y3 = relu(einsum('bfd,bgd,ofg', x, h2, w3) + b3)
    pooled = concat([x1, x2, y3], 1).sum(-1)               # (B, 800)
    out = pooled @ out_w + out_b                           # (B, 400)
    out = batchnorm_train(out, gamma, beta); relu

Strategy: data-parallel over batch (128 rows/core on 8 cores). Each CIN layer
is a GEMM over tokens t=(b,d): z[t,o] = sum_{f,g} V[(f,g),t] * W[(f,g),o]
with V^T[(f,g),t] = x^T[f,t]*h^T[g,t] built on-the-fly on the vector engine
(bf16, fp32 PSUM accumulation). x-side replication tiles are precomputed on
host; h^T comes from PE transposes of each layer's output. Final BN stats are
all-reduced across cores with a tiny collective.
"""

import sys

sys.path.insert(0, "/opt/trn_rl_repo")

import numpy as np
import ml_dtypes

import concourse.bass as bass
import concourse.bacc as bacc
import concourse.mybir as mybir
import concourse.tile as tile
from concourse.masks import make_identity
from concourse.bass_utils import run_bass_kernel_spmd

BF16 = ml_dtypes.bfloat16
FP32 = mybir.dt.float32
BF = mybir.dt.bfloat16

N_CORES = 8
B, F, D = 1024, 39, 32
CL, HALF, OUT = 400, 200, 400
FINAL = 800
EPS = 1e-5

B_LOC = B // N_CORES          # 128 batch rows per core
T_TOT = B_LOC * D             # 4096 tokens per core
TBD = 512                     # tokens per bd-tile
N_BD = T_TOT // TBD           # 8 bd-tiles
N_MS = TBD // 128             # 4 m-slices per bd-tile
NB_TILE = TBD // D            # 16 batch rows per bd-tile

# L1 K-chunking: 13 chunks of 117 rows (3 f's x 39 g's each), K1 = 1521
L1_CHUNKS = 13
L1_ROWS = 117
# L2/L3 K-chunking: per f: A chunk (g 0..127) + B chunk (g 128..199)
GA, GB = 128, 72

# pooled channel chunks: (tag, layer_part, ch0, length, global_row0)
# pooled order = [x1(200) | x2(200) | y3(400)]
POOL_CHUNKS = [
    (0, 1, 0, 128), (1, 1, 128, 72),
    (2, 2, 0, 128), (3, 2, 128, 72),
    (4, 3, 0, 128), (5, 3, 128, 128), (6, 3, 256, 128), (7, 3, 384, 16),
]
POOL_ROW0 = [0, 128, 200, 328, 400, 528, 656, 784]
POOL_LEN = [128, 72, 128, 72, 128, 128, 128, 16]


def _host_prep(x_shard, w1, b1, w2, b2, w3, b3, out_w, out_b):
    """Pure layout prep (transpose/tile/cast) of one core's inputs."""
    # x^T: (F, T) with token t = b*D + d
    xT = np.ascontiguousarray(x_shard.transpose(1, 0, 2).reshape(F, T_TOT))
    # x3[p] = xT[p % 39]  (in0 for L1/L2/L3 f-runs)  -> [N_BD, 117, TBD]
    x3 = np.tile(xT, (3, 1))[:L1_ROWS]
    x3 = np.ascontiguousarray(
        x3.reshape(L1_ROWS, N_BD, TBD).transpose(1, 0, 2)).astype(BF16)
    # xrep[j, p] = xT[3j + p//39]  (L1 in1)  -> [N_BD, 13, 117, TBD]
    xrep = np.empty((L1_CHUNKS, L1_ROWS, T_TOT), np.float32)
    for j in range(L1_CHUNKS):
        for q in range(3):
            xrep[j, 39 * q:39 * (q + 1)] = xT[3 * j + q][None, :]
    xrep = np.ascontiguousarray(
        xrep.reshape(L1_CHUNKS, L1_ROWS, N_BD, TBD).transpose(2, 0, 1, 3)
    ).astype(BF16)
    # repx[f] = xT[f] broadcast to 128 partitions  -> [N_BD, 39, 128, TBD]
    repx = np.broadcast_to(xT[:, None, :], (F, 128, T_TOT))
    repx = np.ascontiguousarray(
        repx.reshape(F, 128, N_BD, TBD).transpose(2, 0, 1, 3)).astype(BF16)

    # weights, K=(f,g) flattened f-major
    w1r = np.ascontiguousarray(
        w1.transpose(1, 2, 0).reshape(F * F, CL)).astype(BF16)
    def _wpack(w):
        wr = w.transpose(1, 2, 0).reshape(F, HALF, CL)  # [f, g, o]
        wp = np.zeros((F, 128, 2 * CL), np.float32)
        wp[:, :, :CL] = wr[:, :GA]
        wp[:, :GB, CL:] = wr[:, GA:]
        return wp.astype(BF16)
    w2r = _wpack(w2)
    w3r = _wpack(w3)
    worc = np.zeros((8, 128, OUT), np.float32)
    for k in range(8):
        worc[k, :POOL_LEN[k]] = out_w[POOL_ROW0[k]:POOL_ROW0[k] + POOL_LEN[k]]
    worc = worc.astype(BF16)
    biases = np.stack([b1, b2, b3, out_b]).astype(BF16)  # [4, 400]
    return dict(x3=x3, xrep=xrep, repx=repx, w1r=w1r, w2r=w2r, w3r=w3r,
                worc=worc, biases=biases)


def _build_bass():
    nc = bacc.Bacc()
    P = {}
    P["x3"] = nc.declare_dram_parameter("x3", [N_BD, L1_ROWS, TBD], BF, isOutput=False)
    P["xrep"] = nc.declare_dram_parameter("xrep", [N_BD, L1_CHUNKS, L1_ROWS, TBD], BF, isOutput=False)
    P["repx"] = nc.declare_dram_parameter("repx", [N_BD, F, 128, TBD], BF, isOutput=False)
    P["w1r"] = nc.declare_dram_parameter("w1r", [F * F, CL], BF, isOutput=False)
    P["w2r"] = nc.declare_dram_parameter("w2r", [F, 128, 2 * CL], BF, isOutput=False)
    P["w3r"] = nc.declare_dram_parameter("w3r", [F, 128, 2 * CL], BF, isOutput=False)
    P["worc"] = nc.declare_dram_parameter("worc", [8, 128, OUT], BF, isOutput=False)
    P["biases"] = nc.declare_dram_parameter("biases", [4, OUT], BF, isOutput=False)
    P["gamma"] = nc.declare_dram_parameter("gamma", [1, OUT], FP32, isOutput=False)
    P["beta"] = nc.declare_dram_parameter("beta", [1, OUT], FP32, isOutput=False)
    out_d = nc.declare_dram_parameter("out", [B_LOC, OUT], FP32, isOutput=True)

    MULT = mybir.AluOpType.mult
    ADD = mybir.AluOpType.add
    SUB = mybir.AluOpType.subtract
    RELU = mybir.ActivationFunctionType.Relu
    COPY = mybir.ActivationFunctionType.Copy
    SQRT = mybir.ActivationFunctionType.Sqrt

    with tile.TileContext(nc) as tc:
        with (
            tc.tile_pool(name="wpool", bufs=1) as wpool,
            tc.tile_pool(name="consts", bufs=1) as consts,
            tc.tile_pool(name="rpool", bufs=3) as rpool,
            tc.tile_pool(name="xpool", bufs=2) as xpool,
            tc.tile_pool(name="vpool", bufs=3) as vpool,
            tc.tile_pool(name="hpool", bufs=2) as hpool,
            tc.tile_pool(name="ypool", bufs=4) as ypool,
            tc.tile_pool(name="spool", bufs=1) as spool,
            tc.tile_pool(name="psy", bufs=5, space="PSUM") as psy,
            tc.tile_pool(name="pst", bufs=2, space="PSUM") as pst,
            tc.tile_pool(name="psp", bufs=1, space="PSUM") as psp,
            tc.tile_pool(name="dram", bufs=1, space="DRAM") as dram,
        ):
            # ---- constants ----
            ident = consts.tile([128, 128], BF, tag="ident", name="ident")
            make_identity(nc, ident)
            ones_bf = consts.tile([1, 128], BF, tag="ones_bf", name="ones_bf")
            nc.vector.memset(ones_bf[:], 1.0)
            ones_col = consts.tile([128, 1], FP32, tag="ones_col", name="ones_col")
            nc.vector.memset(ones_col[:], 1.0)
            ones_row_f = consts.tile([1, 128], FP32, tag="ones_row_f", name="ones_row_f")
            nc.vector.memset(ones_row_f[:], 1.0)
            # A4[t, j] = 1 if t//32 == j (d-sum aggregation within an m-slice)
            a4 = consts.tile([128, 4], BF, tag="a4", name="a4")
            nc.vector.memset(a4[:], 0.0)
            for j in range(4):
                nc.vector.memset(a4[32 * j:32 * (j + 1), j:j + 1], 1.0)
            gamma_sb = consts.tile([1, OUT], FP32, tag="gamma", name="gamma")
            nc.sync.dma_start(gamma_sb[:], P["gamma"][:])
            beta_sb = consts.tile([1, OUT], FP32, tag="beta", name="beta")
            nc.sync.dma_start(beta_sb[:], P["beta"][:])
            brow = []
            for l in range(4):
                t = consts.tile([1, OUT], BF, tag=f"brow{l}", name=f"brow{l}")
                nc.sync.dma_start(t[:], P["biases"][l:l + 1, :])
                brow.append(t)

            w2c, w3c = [], []

            # pooled^T accumulator chunks, filled 16 b-columns per bd-tile
            pooledT = []
            for k in range(8):
                t = spool.tile([128, B_LOC], BF, tag=f"pooledT{k}", name=f"pooledT{k}")
                pooledT.append(t)

            # ---- main loop over bd-tiles ----
            for c in range(N_BD):
                # x-side tiles for this bd-tile
                x3c = xpool.tile([L1_ROWS, TBD], BF, tag="x3c", name="x3c", bufs=1)
                nc.sync.dma_start(x3c[:], P["x3"][c])
                xrepc = []
                XQ = (1, 6, 6)
                j0 = 0
                for q, nj in enumerate(XQ):
                    t = xpool.tile([L1_ROWS, 6 * TBD], BF, tag="xrepc",
                                   name=f"xrepc{q}", bufs=3)
                    src_ap = P["xrep"][c, j0:j0 + nj].rearrange("j p t -> p j t")
                    dst_ap = t[:, 0:nj * TBD].rearrange("p (j t) -> p j t", j=nj)
                    nc.sync.dma_start(dst_ap, src_ap)
                    for j in range(nj):
                        xrepc.append(t[:, j * TBD:(j + 1) * TBD])
                    j0 += nj
                pooled_ps = psp.tile([128, 128], FP32, tag="pooled", name="pooled")
                w1c = []
                for j in range(L1_CHUNKS):
                    t = wpool.tile([L1_ROWS, CL], BF, tag="w1s", name=f"w1s{c}_{j}", bufs=5)
                    nc.sync.dma_start(t[:], P["w1r"][L1_ROWS * j:L1_ROWS * (j + 1), :])
                    w1c.append(t)

                hT_A = hT_B = None
                for layer in (1, 2, 3):
                    # --- build V^T chunks and accumulate the layer GEMM ---
                    psum_y = [psy.tile([128, CL], FP32, tag="psy", name=f"psy{layer}_{c}_{m}") for m in range(N_MS)]
                    if layer == 1:
                        nck = L1_CHUNKS
                        for j in range(nck):
                            vt = vpool.tile([L1_ROWS, TBD], BF, tag="vt", name="vt")
                            nc.vector.tensor_tensor(vt[:], x3c[:], xrepc[j], MULT)
                            for m in range(N_MS):
                                nc.tensor.matmul(
                                    psum_y[m][:], vt[:, 128 * m:128 * (m + 1)],
                                    w1c[j][:], start=(j == 0), stop=False)
                    else:
                        wc = w2c if layer == 2 else w3c
                        repq = []
                        for q in range((F + 3) // 4):
                            nf = min(4, F - 4 * q)
                            t = rpool.tile([128, 4 * TBD], BF, tag="repc",
                                           name=f"repc{layer}_{q}")
                            src_ap = P["repx"][c, 4 * q:4 * q + nf].rearrange(
                                "f p t -> p f t")
                            dst_ap = t[:, 0:nf * TBD].rearrange(
                                "p (f t) -> p f t", f=nf)
                            nc.sync.dma_start(dst_ap, src_ap)
                            repq.append(t)
                        repc = [repq[f // 4][:, (f % 4) * TBD:(f % 4 + 1) * TBD]
                                for f in range(F)]
                        for f in range(F):
                            if c == 0:
                                # lazy, interleaved first-use weight loads
                                wl = P["w2r"] if layer == 2 else P["w3r"]
                                wp = "w2" if layer == 2 else "w3"
                                tw = wpool.tile([128, 2 * CL], BF, tag=f"{wp}_{f}", name=f"{wp}_{f}")
                                nc.sync.dma_start(tw[:], wl[f])
                                wc.append(tw)
                            va = vpool.tile([GA, TBD], BF, tag="vt", name="vt")
                            nc.vector.tensor_tensor(
                                va[:], hT_A[:], repc[f], MULT)
                            for m in range(N_MS):
                                nc.tensor.matmul(
                                    psum_y[m][:], va[:, 128 * m:128 * (m + 1)],
                                    wc[f][:, 0:CL], start=(f == 0), stop=False)
                            vb = vpool.tile([GB, TBD], BF, tag="vtb", name="vtb")
                            nc.vector.tensor_tensor(
                                vb[:], hT_B[0:GB, :], repc[f][0:GB, 0:TBD], MULT)
                            for m in range(N_MS):
                                nc.tensor.matmul(
                                    psum_y[m][:], vb[:, 128 * m:128 * (m + 1)],
                                    wc[f][0:GB, CL:2 * CL], start=False, stop=False)
                    # bias row (K=1 matmul), closes the accumulation group
                    for m in range(N_MS):
                        nc.tensor.matmul(
                            psum_y[m][:], ones_bf[:], brow[layer - 1][:],
                            start=False, stop=True)

                    # --- drain with relu; produce h^T for next layer ---
                    if layer < 3:
                        hT_A = hpool.tile([GA, TBD], BF, tag="hTa", name="hTa")
                        hT_B = hpool.tile([GB, TBD], BF, tag="hTb", name="hTb")
                    y_sb = []
                    for m in range(N_MS):
                        y = ypool.tile([128, CL], BF, tag="ysb", name="ysb")
                        nc.scalar.activation(y[:], psum_y[m][:], RELU)
                        y_sb.append(y)
                        if layer < 3:
                            ta = pst.tile([128, 128], BF, tag="tp", name="tp")
                            nc.tensor.transpose(
                                ta[:], y[:, HALF:HALF + GA], ident[:])
                            nc.scalar.copy(
                                hT_A[:, 128 * m:128 * (m + 1)], ta[:])
                            tb = pst.tile([GB, 128], BF, tag="tp", name="tp")
                            nc.tensor.transpose(
                                tb[:], y[:, HALF + GA:CL], ident[:])
                            nc.scalar.copy(
                                hT_B[:, 128 * m:128 * (m + 1)], tb[:])
                    # --- pooled d-sums for this layer's x-part / y3 ---
                    for (k, lp, ch0, ln) in POOL_CHUNKS:
                        if lp != layer:
                            continue
                        for m in range(N_MS):
                            nc.tensor.matmul(
                                pooled_ps[0:ln, 16 * k + 4 * m:16 * k + 4 * (m + 1)],
                                y_sb[m][:, ch0:ch0 + ln], a4[:],
                                start=True, stop=True)
                # drain pooled psum into persistent pooled^T chunks
                for k in range(8):
                    ln = POOL_LEN[k]
                    nc.scalar.copy(
                        pooledT[k][0:ln, NB_TILE * c:NB_TILE * (c + 1)],
                        pooled_ps[0:ln, 16 * k:16 * (k + 1)])

                # --- per-bd-tile final GEMM on this tile's 16 batch rows ---
                if c == 0:
                    worc_sb = []
                    for k in range(8):
                        t = wpool.tile([128, OUT], BF, tag=f"wor{k}", name=f"wor{k}")
                        nc.sync.dma_start(t[:], P["worc"][k])
                        worc_sb.append(t)
                    out_sb = spool.tile([B_LOC, OUT], FP32, tag="out_sb", name="out_sb")
                    stats_acc = spool.tile([1, 2 * OUT], FP32, tag="stats_acc", name="stats_acc")
                fo_ps = pst.tile([NB_TILE, OUT], FP32, tag="tp", name=f"fo_ps{c}")
                for k in range(8):
                    nc.tensor.matmul(
                        fo_ps[:], pooledT[k][0:POOL_LEN[k], NB_TILE * c:NB_TILE * (c + 1)],
                        worc_sb[k][0:POOL_LEN[k], :], start=(k == 0), stop=False)
                nc.tensor.matmul(fo_ps[:], ones_bf[0:1, 0:NB_TILE], brow[3][:],
                                 start=False, stop=True)
                o_c = spool.tile([NB_TILE, OUT], FP32, tag="o_c", name="o_c", bufs=1)
                nc.scalar.copy(o_c[:], fo_ps[:])
                nc.sync.dma_start(out_sb[NB_TILE * c:NB_TILE * (c + 1), :], o_c[:])
                sq_c = spool.tile([NB_TILE, OUT], FP32, tag="sq_c", name="sq_c", bufs=1)
                nc.scalar.square(sq_c[:], o_c[:])
                st_ps = pst.tile([1, OUT], FP32, tag="tp", name=f"st_ps{c}")
                nc.tensor.matmul(st_ps[:], ones_col[0:NB_TILE, :], o_c[:],
                                 start=True, stop=True)
                sq_ps = pst.tile([1, OUT], FP32, tag="tp", name=f"sq_ps{c}")
                nc.tensor.matmul(sq_ps[:], ones_col[0:NB_TILE, :], sq_c[:],
                                 start=True, stop=True)
                if c == 0:
                    nc.scalar.copy(stats_acc[0:1, 0:OUT], st_ps[:])
                    nc.scalar.copy(stats_acc[0:1, OUT:2 * OUT], sq_ps[:])
                else:
                    tgt = stats7 if c == N_BD - 1 else stats_acc
                    if c == N_BD - 1:
                        nc.scalar.copy(stats7[0:1, 0:OUT], st_ps[:])
                        nc.scalar.copy(stats7[0:1, OUT:2 * OUT], sq_ps[:])
                    else:
                        nc.vector.tensor_tensor(
                            stats_acc[0:1, 0:OUT], stats_acc[0:1, 0:OUT], st_ps[:], ADD)
                        nc.vector.tensor_tensor(
                            stats_acc[0:1, OUT:2 * OUT], stats_acc[0:1, OUT:2 * OUT],
                            sq_ps[:], ADD)
                if c == N_BD - 2:
                    # early collective over the first 112 rows' stats; its
                    # latency hides under bd-tile 7 compute
                    nc.gpsimd.dma_start(cc_in1[:], stats_acc[:])
                    nc.gpsimd.collective_compute(
                        "AllReduce", ADD,
                        replica_groups=[list(range(N_CORES))],
                        ins=[cc_in1.opt()], outs=[cc_out1.opt()])
                    nc.gpsimd.dma_start(gst1[:], cc_out1[:])
                if c == 0:
                    stats7 = spool.tile([1, 2 * OUT], FP32, tag="stats7", name="stats7")
                    cc_in1 = dram.tile([1, 2 * OUT], FP32, tag="", name="cc_in1")
                    cc_out1 = dram.tile([1, 2 * OUT], FP32, tag="", name="cc_out1")
                    cc_in2 = dram.tile([1, 2 * OUT], FP32, tag="", name="cc_in2")
                    cc_out2 = dram.tile([1, 2 * OUT], FP32, tag="", name="cc_out2")
                    gst1 = spool.tile([1, 2 * OUT], FP32, tag="gst1", name="gst1")

            # ---- second-phase collective: last bd-tile's stats ----
            nc.gpsimd.dma_start(cc_in2[:], stats7[:])
            nc.gpsimd.collective_compute(
                "AllReduce", ADD,
                replica_groups=[list(range(N_CORES))],
                ins=[cc_in2.opt()], outs=[cc_out2.opt()])
            gst = spool.tile([1, 2 * OUT], FP32, tag="stats_acc", name="gst")
            nc.gpsimd.dma_start(gst[:], cc_out2[:])
            nc.vector.tensor_tensor(gst[:], gst[:], gst1[:], ADD)

            # mean/var/scale/shift on partition 0
            mean = spool.tile([1, OUT], FP32, tag="mean", name="mean")
            nc.scalar.mul(mean[:], gst[0:1, 0:OUT], 1.0 / B)
            msq = spool.tile([1, OUT], FP32, tag="scratch", name="msq", bufs=2)
            nc.vector.tensor_tensor(msq[:], mean[:], mean[:], MULT)
            var = spool.tile([1, OUT], FP32, tag="scratch", name="var", bufs=2)
            nc.vector.scalar_tensor_tensor(
                var[:], gst[0:1, OUT:2 * OUT], 1.0 / B, msq[:], MULT, SUB)
            epsc = spool.tile([1, 1], FP32, tag="epsc", name="epsc")
            nc.vector.memset(epsc[:], EPS)
            stdv = spool.tile([1, OUT], FP32, tag="scratch", name="stdv", bufs=2)
            nc.scalar.activation(stdv[:], var[:], SQRT, bias=epsc[:])
            inv = spool.tile([1, OUT], FP32, tag="scratch", name="inv", bufs=2)
            nc.vector.reciprocal(inv[:], stdv[:])
            scal = spool.tile([1, OUT], FP32, tag="scal", name="scal")
            nc.vector.tensor_tensor(scal[:], inv[:], gamma_sb[:], MULT)
            tmp = spool.tile([1, OUT], FP32, tag="scratch", name="tmp", bufs=2)
            nc.vector.scalar_tensor_tensor(
                tmp[:], mean[:], -1.0, scal[:], MULT, MULT)
            shift = spool.tile([1, OUT], FP32, tag="shift", name="shift")
            nc.vector.tensor_tensor(shift[:], tmp[:], beta_sb[:], ADD)

            # broadcast scale/shift across partitions via K=1 matmuls
            bc_scale = psy.tile([128, OUT], FP32, tag="psy", name="psy")
            nc.tensor.matmul(bc_scale[:], ones_row_f[:], scal[:],
                             start=True, stop=True)
            bc_shift = psy.tile([128, OUT], FP32, tag="psy", name="psy")
            nc.tensor.matmul(bc_shift[:], ones_row_f[:], shift[:],
                             start=True, stop=True)
            t1 = spool.tile([128, OUT], FP32, tag="scratch", name="t1", bufs=2)
            nc.vector.tensor_tensor(t1[:], out_sb[:], bc_scale[:], MULT)
            t2 = spool.tile([128, OUT], FP32, tag="scratch", name="t2", bufs=2)
            nc.vector.tensor_tensor(t2[:], t1[:], bc_shift[:], ADD)
            outf = spool.tile([128, OUT], FP32, tag="scratch", name="outf", bufs=2)
            nc.scalar.activation(outf[:], t2[:], RELU)
            nc.sync.dma_start(out_d[:], outf[:])

    nc.compile()
    return nc


_NC_CACHE = None


def kernel(x, w1, b1, w2, b2, w3, b3, out_w, out_b, gamma, beta):
    global _NC_CACHE
    x = np.asarray(x, np.float32)
    shards = [
        _host_prep(x[B_LOC * i:B_LOC * (i + 1)],
                   np.asarray(w1, np.float32), np.asarray(b1, np.float32),
                   np.asarray(w2, np.float32), np.asarray(b2, np.float32),
                   np.asarray(w3, np.float32), np.asarray(b3, np.float32),
                   np.asarray(out_w, np.float32), np.asarray(out_b, np.float32))
        for i in range(N_CORES)
    ]
    gm = np.asarray(gamma, np.float32).reshape(1, OUT)
    bt = np.asarray(beta, np.float32).reshape(1, OUT)
    in_maps = [dict(s, gamma=gm, beta=bt) for s in shards]

    if _NC_CACHE is None:
        _NC_CACHE = _build_bass()
    res = run_bass_kernel_spmd(_NC_CACHE, in_maps, core_ids=list(range(N_CORES)))
    return np.concatenate(
        [res.results[i]["out"] for i in range(N_CORES)], axis=0
    ).astype(np.float32)

